# revision 1
# baseline (speedup 1.0000x reference)
"""EuclideanDeconf kernel for 8x TRN2 NeuronCores.

Computes out[b, c] = (2/D) * x @ W.T - ||x||^2/D - ||W||^2/D
for x [16384, 1024] f32, W [2048, 1024] f32 -> out [16384, 2048] f32.

Sharding: data-parallel over the batch dim. Each of the 8 cores gets 2048
rows of x (passed pre-transposed as xT [1024, 2048] f32) and the full W
(passed pre-transposed, scaled by 16 and e4m3-cast as wT [1024, 2048]).
The host does layout-only work (transpose / cast / shard / concat); all
FLOPs (matmul, row/col norms, combine) run on device.

Numerics (default fp8 mode): the cross term's magnitude is only ~0.003 of
the ~1.0 output (which is dominated by -||x||^2/D), so e4m3 rounding of the
matmul operands contributes only ~1e-4 relative error to the output. x2 is
computed on-device in fp32 from the fp32 x (the dominant term, kept exact);
w2 from e4m3 W (w2 is ~0.002, so its rounding is ~1e-5 absolute). Measured
vs the fp32 reference: max rel err 6.2e-4, norm rel err 1.0e-4. The bf16
mode (K_MM=bf16) gives max rel err 4e-5 at ~20% more runtime.

Engine assignment (per core, fp8 mode, HW ~122us):
  PE:     256 e4m3 DoubleRow matmuls (K=256 per op; the 8.6 GFLOP core of
          the op) + 32 w2-reduce + 4 w2-replicate + 16 tiny x2-dot matmuls
          + warmup (dummy matmuls so the PE HAM clock-gate releases early)
  ACT:    W^2 squares, psum->sbuf copy-outs, epilogue pass 1:
          t = (2/(16D))*psum - x2[b]  (scale + per-partition bias)
  DVE:    x f32->fp8 casts, x2 k-add-trees, epilogue pass 2: y = t - w2[c]
  GPSIMD: x^2 squares only
  DMA:    everything on the SP (sync) HWDGE ring; x chunk 0 first, W second

All engines execute their queues strictly in program order, so emission
order is the schedule: the w2 chain (W DMA -> wsq -> reduce -> replicate)
is emitted before chunk 0's b-tile groups (its DVE copies must not queue
behind epilogue adds that depend on them), and x2 columns are produced
per-b-tile so ACT can drain PSUM as soon as each accumulation closes.
Variants measured and rejected on HW: rank-1 w2-fold into PSUM (+13.6us
PE, made PE the bottleneck again: 136us), SWDGE cast-DMA for x8 (SWDGE
cast path is ~5x too slow: 153us), x2 trees on gpsimd (per-op overhead:
152us), chunk-0 matmuls emitted before the w2 chain (130us).
"""

import numpy as np
import ml_dtypes

# Problem constants (hardcoded; kernel.py must be self-contained).
B, D, C = 16384, 1024, 2048
NCORES = 8
BSH = B // NCORES  # 2048 rows of x per core
P = 128            # partitions
KT = D // P        # 8 contraction tiles
BCH = 512          # b-chunk (columns of xT loaded per DMA)

_CACHE = {}

import os as _os

# "bf16": plain bf16 matmuls (max rel err ~4e-5, HW ~164us)
# "fp8": e4m3 + DoubleRow matmuls (max rel err ~6e-4, HW ~122us)
MM_MODE = _os.environ.get("K_MM", "fp8")


def _build_nc():
    import concourse.tile as tile
    import concourse.mybir as mybir
    import concourse.bass as bass
    from concourse import bacc

    f32 = mybir.dt.float32
    bf16 = mybir.dt.bfloat16
    PSUM = bass.MemorySpace.PSUM
    Identity = mybir.ActivationFunctionType.Identity
    Copy = mybir.ActivationFunctionType.Copy
    MULT = mybir.AluOpType.mult
    ADD = mybir.AluOpType.add

    fp8 = MM_MODE == "fp8"
    mdt = mybir.dt.float8e4 if fp8 else bf16   # matmul operand dtype
    # In fp8 mode W is host-prescaled by 16 (keeps values out of the e4m3
    # subnormal range); the epilogue scale folds the 1/16 back out.
    cross_scale = 2.0 / D / (16.0 if fp8 else 1.0)
    w2_scale = 1.0 / D / (256.0 if fp8 else 1.0)
    DR = mybir.MatmulPerfMode.DoubleRow if fp8 else None

    nc = bacc.Bacc(
        "TRN2",
        target_bir_lowering=False,
        debug=False,
        enable_asserts=False,
    )
    xT = nc.dram_tensor("xT", [D, BSH], f32, kind="ExternalInput").ap()
    wT = nc.dram_tensor("wT", [D, C], mdt, kind="ExternalInput").ap()
    y = nc.dram_tensor("y", [BSH, C], f32, kind="ExternalOutput").ap()

    with tile.TileContext(nc) as tc:
        with (
            tc.tile_pool(name="consts", bufs=1) as cpool,
            tc.tile_pool(name="wpool", bufs=1) as wpool,
            tc.tile_pool(name="xpool", bufs=2) as xpool,
            tc.tile_pool(name="xsqpool", bufs=3) as xsqpool,
            tc.tile_pool(name="epool", bufs=8) as epool,
            tc.tile_pool(name="ypool", bufs=3) as ypool,
            tc.tile_pool(name="spool", bufs=8) as spool,
            tc.tile_pool(name="pmain", bufs=3, space=PSUM) as pmain,
            tc.tile_pool(name="psmall", bufs=1, space=PSUM) as psmall,
        ):
            negones_f = cpool.tile([P, 1], f32)
            nc.gpsimd.memset(negones_f[:], -1.0)
            negones_b = cpool.tile([P, 1], bf16)
            nc.gpsimd.memset(negones_b[:], -1.0)
            ones1_b = cpool.tile([1, P], bf16)
            nc.gpsimd.memset(ones1_b[:], 1.0)
            warm = cpool.tile([1, 1], f32)
            # touch ACT early so its function-table DMA (~2.7us) is off the
            # critical path by the time the first epilogue runs
            nc.scalar.activation(warm[:], negones_f[0:1, 0:1], Identity,
                                 bias=0.0, scale=1.0)

            # ---- PE warmup: dummy matmuls so HAM un-throttles (and the PE
            # is at 2.4 GHz) by the time real work arrives ----
            warm_b = cpool.tile([P, 512], bf16)
            nc.gpsimd.memset(warm_b[:], 0.0)
            warm_ps = psmall.tile([P, 512], f32, tag="w2ps", bufs=1)
            for _ in range(20):
                nc.tensor.matmul(warm_ps[:], warm_b[:, 0:P], warm_b[:],
                                 start=True, stop=True)

            wbf = wpool.tile([P, KT, C], mdt)
            wTr = wT.rearrange("(k p) c -> p k c", p=P)

            y_bufs = {}

            def btile_matmuls(jg, xbf, jl):
                """Issue the 32 accumulating matmuls for one 128-row b-tile."""
                y_t = ypool.tile([P, C], f32, tag="y_t", name=f"y_t{jg}")
                ps0 = pmain.tile([P, 1024], f32, tag="ps", name=f"ps{jg}a")
                ps1 = pmain.tile([P, 1024], f32, tag="ps", name=f"ps{jg}b")
                pss = (ps0, ps0, ps1, ps1)
                if fp8:
                    for k2 in range(KT // 2):
                        lhsT = xbf[:, 2 * k2:2 * k2 + 2, jl * P:(jl + 1) * P]
                        for cj in range(4):
                            nc.tensor.matmul(
                                pss[cj][:, (cj % 2) * 512:(cj % 2) * 512 + 512],
                                lhsT,
                                wbf[:, 2 * k2:2 * k2 + 2, cj * 512:(cj + 1) * 512],
                                start=(k2 == 0),
                                stop=(k2 == KT // 2 - 1),
                                perf_mode=DR,
                            )
                else:
                    for k in range(KT):
                        lhsT = xbf[:, k, jl * P:(jl + 1) * P]
                        for cj in range(4):
                            nc.tensor.matmul(
                                pss[cj][:, (cj % 2) * 512:(cj % 2) * 512 + 512],
                                lhsT,
                                wbf[:, k, cj * 512:(cj + 1) * 512],
                                start=(k == 0),
                                stop=(k == KT - 1),
                            )
                y_bufs[jg] = (y_t, ps0, ps1)

            def x2_tree(xsq, tag):
                """k-add-tree for one b-tile's x^2 partials (DVE)."""
                t4 = xsqpool.tile([P, 4, P], f32, tag="t4", name=f"t4_{tag}")
                nc.vector.tensor_tensor(t4[:], xsq[:, 0:4, :], xsq[:, 4:8, :],
                                        op=ADD)
                t2 = xsqpool.tile([P, 2, P], f32, tag="t2", name=f"t2_{tag}")
                nc.vector.tensor_tensor(t2[:], t4[:, 0:2, :], t4[:, 2:4, :],
                                        op=ADD)
                t1 = xsqpool.tile([P, P], f32, tag="t1", bufs=8,
                                  name=f"t1_{tag}")
                nc.vector.tensor_tensor(t1[:], t2[:, 0, :], t2[:, 1, :], op=ADD)
                return t1

            def x2_col(t1, tag):
                """x2 column (-sum(x^2)/D) for one b-tile: PE dot + ACT copy."""
                x2ps = psmall.tile([P, 1], f32, tag="x2ps", bufs=1,
                                   name=f"x2ps{tag}")
                nc.tensor.matmul(x2ps[:], t1[:], negones_f[:],
                                 start=True, stop=True)
                x2c = spool.tile([P, 1], f32, tag="x2c", name=f"x2c{tag}")
                # copy-out on ACT (idle early; DVE is busy with casts/wsq)
                nc.scalar.activation(x2c[:], x2ps[:], Copy, bias=0.0,
                                     scale=1.0 / D)
                return x2c

            def prep(ch):
                """DMA + cast + x^2 squares + k-trees for one chunk (all the
                per-chunk work with no PSUM/epilogue dependencies), emitted
                ahead of compute so the in-order DVE/GPSIMD queues never make
                the PE wait at a chunk boundary."""
                xTr = xT[:, ch * BCH:(ch + 1) * BCH].rearrange(
                    "(k p) b -> p k b", p=P
                )
                xf = xpool.tile([P, KT, BCH], f32, tag="xf", name=f"xf{ch}")
                xbf = xpool.tile([P, KT, BCH], mdt, tag="xbf",
                                 bufs=(3 if fp8 else 2), name=f"xbf{ch}")
                if ch == 0:
                    for k in range(KT):
                        nc.sync.dma_start(xf[:, k, :], xTr[:, k, :])
                        nc.vector.tensor_copy(xbf[:, k, :], xf[:, k, :])
                else:
                    nc.sync.dma_start(xf[:], xTr)
                    nc.vector.tensor_copy(xbf[:], xf[:])
                t1s = []
                for jj in range(BCH // P):
                    sl = slice(jj * P, (jj + 1) * P)
                    xsq = xsqpool.tile([P, KT, P], f32, tag="xsq",
                                       name=f"xsq{ch}_{jj}")
                    nc.gpsimd.tensor_tensor(xsq[:], xf[:, :, sl],
                                            xf[:, :, sl], op=MULT)
                    t1s.append(x2_tree(xsq, f"c{ch}_{jj}"))
                return xbf, t1s

            def compute(ch, xbf, t1s):
                for jj in range(BCH // P):
                    j = ch * (BCH // P) + jj
                    btile_matmuls(j, xbf, jj)
                    x2c = x2_col(t1s[jj], f"c{ch}_{jj}")
                    btile_epilogue(j, x2c, w2rep, split=(j == BSH // P - 1))

            def btile_epilogue(jg, x2c, w2rep, split=False):
                y_t, ps0, ps1 = y_bufs.pop(jg)
                for h, psh in enumerate((ps0, ps1)):
                    ysl = y_t[:, h * 1024:(h + 1) * 1024]
                    t = epool.tile([P, 1024], f32, tag="t", name=f"t{jg}_{h}")
                    # t = cross_scale*psum - x2  (scale + per-partition bias)
                    nc.scalar.activation(t[:], psh[:], Identity,
                                         bias=x2c[:], scale=cross_scale)
                    # y = t - w2  (w2rep already negated)
                    nc.vector.tensor_add(
                        ysl, t[:], w2rep[:, h * 1024:(h + 1) * 1024]
                    )
                    if split:
                        # last b-tile: store each half as soon as it's ready
                        # so the final DMA overlaps the second half's epilogue
                        nc.sync.dma_start(
                            y[jg * P:(jg + 1) * P, h * 1024:(h + 1) * 1024],
                            ysl,
                        )
                if not split:
                    nc.sync.dma_start(y[jg * P:(jg + 1) * P, :], y_t[:])

            # ---- chunk 0 prep first (x pieces lead on the sync ring so
            # casts/squares start immediately), then W ----
            xbf0, t1s0 = prep(0)
            for k in range(KT):
                nc.sync.dma_start(wbf[:, k, :], wTr[:, k, :])

            # ---- w2: squares, partition reduce on PE ----
            # fp8: squares on ACT (DVE is the scarce engine); w2row becomes a
            #      bf16 row folded into each b-tile's PSUM via rank-1 matmuls.
            # bf16: squares on DVE; w2row replicated to [128, C] f32 for the
            #      DVE epilogue-subtract pass.
            wsq = wpool.tile([P, KT, C], bf16)
            Square = mybir.ActivationFunctionType.Square
            for k in range(KT):
                if fp8:
                    nc.scalar.activation(wsq[:, k, :], wbf[:, k, :], Square)
                else:
                    nc.vector.tensor_tensor(wsq[:, k, :], wbf[:, k, :],
                                            wbf[:, k, :], op=MULT)
            w2row = wpool.tile([1, C], bf16)
            for cj in range(C // 512):
                w2ps = psmall.tile([1, 512], f32, tag="w2ps", bufs=1,
                                   name=f"w2ps{cj}")
                for k in range(KT):
                    nc.tensor.matmul(
                        w2ps[:],
                        negones_b[:],
                        wsq[:, k, cj * 512:(cj + 1) * 512],
                        start=(k == 0),
                        stop=(k == KT - 1),
                    )
                # w2row = -sum(W^2)/D (bf16 row; its values are ~2e-3 so
                # bf16 rounding is ~1e-5 absolute on the output)
                nc.scalar.activation(w2row[:, cj * 512:(cj + 1) * 512],
                                     w2ps[:], Copy, bias=0.0, scale=w2_scale)
            w2rep = wpool.tile([P, C], f32)
            for cj in range(C // 512):
                w2rp = psmall.tile([P, 512], f32, tag="w2ps", bufs=1,
                                   name=f"w2rp{cj}")
                nc.tensor.matmul(w2rp[:], ones1_b[:],
                                 w2row[:, cj * 512:(cj + 1) * 512],
                                 start=True, stop=True)
                nc.scalar.activation(w2rep[:, cj * 512:(cj + 1) * 512],
                                     w2rp[:], Copy, bias=0.0, scale=1.0)

            # ---- pipelined chunks: prep runs 2 chunks ahead of compute
            # so chunk-boundary cast/square/tree latency never stalls PE ----
            chunk_state = {0: (xbf0, t1s0)}
            chunk_state[1] = prep(1)
            for ch in range(BSH // BCH):
                xbf, t1s = chunk_state.pop(ch)
                compute(ch, xbf, t1s)
                if ch + 2 < BSH // BCH:
                    chunk_state[ch + 2] = prep(ch + 2)

    nc.compile()
    return nc


def _get_nc():
    if "nc" not in _CACHE:
        _CACHE["nc"] = _build_nc()
    return _CACHE["nc"]


def _prep_inputs(x, W):
    x = np.ascontiguousarray(x, dtype=np.float32)
    W = np.ascontiguousarray(W, dtype=np.float32)
    if MM_MODE == "fp8":
        # prescale by 16 to keep W out of the e4m3 subnormal range; the
        # kernel's epilogue scale folds the 1/16 back out
        wT = np.ascontiguousarray(W.T * np.float32(16.0)).astype(
            ml_dtypes.float8_e4m3
        )
    else:
        wT = np.ascontiguousarray(W.T).astype(ml_dtypes.bfloat16)
    in_maps = []
    for i in range(NCORES):
        xT_i = np.ascontiguousarray(x[i * BSH:(i + 1) * BSH, :].T)
        in_maps.append({"xT": xT_i, "wT": wT})
    return in_maps


def run(x, W, trace=False, **trace_kwargs):
    """Run on the 8 cores; returns (out [B, C] f32, BassKernelResults)."""
    from concourse import bass_utils

    nc = _get_nc()
    in_maps = _prep_inputs(x, W)
    res = bass_utils.run_bass_kernel_spmd(
        nc, in_maps, core_ids=list(range(NCORES)), trace=trace, **trace_kwargs
    )
    out = np.concatenate([r["y"] for r in res.results], axis=0)
    return out, res


def kernel(x, W, task_id=None, **_unused):
    out, _ = run(np.asarray(x), np.asarray(W), trace=False)
    return out

